# revision 10
# baseline (speedup 1.0000x reference)
"""Sliding-window GQA attention (maxtext-style) on 8 Trainium2 NeuronCores.

Problem (hardcoded): B=4, S=2048, NQ=8, NKV=2, D=128, window=1024,
logit soft-cap 50, causal. decoder_segment_ids is all-ones per the input
spec, so the segment mask reduces to causal+window and is not computed on
device.

Sharding: one core per (batch b, kv-head h) pair -> 8 cores, no
collectives. Each core runs sliding-window flash attention for its 4
query heads against its single shared K/V head.

Per-core layout ("layout B"): logits are computed transposed,
L[s, q] = (K Q^T)^T tiles, so the exp'd probabilities P[s, q] feed the
P->V matmul directly as the moving operand (lhsT = V[s, d] natural,
out = O^T[d, q]) with no per-tile P transposes. Softmax needs no
max-subtraction because the tanh soft-cap bounds logits to +-50.
Band masking (causal diagonal + far window edge) is applied by
accumulating a -1e30 rank-128 bias product into the logits PSUM, which
the tanh saturates to -1 -> exp gives e^-50 ~ 2e-22 (negligible).
Row sums ride on a [1, q] ones-matmul accumulated alongside O^T; the
final normalize is a reciprocal + broadcast-matmul + vector multiply.
"""

import math
from contextlib import ExitStack

import numpy as np

import concourse.bass as bass
import concourse.tile as tile
from concourse import bacc, mybir
from concourse.bass_utils import run_bass_kernel_spmd

F32 = mybir.dt.float32
F32R = mybir.dt.float32r
AFT = mybir.ActivationFunctionType

# Full-size problem constants
B, S, NQ, NKV, D = 4, 2048, 8, 2, 128
G = NQ // NKV  # 4 query heads per kv head
S_TILES = S // 128  # 16
W_TILES = 1024 // 128  # 8 (sliding window in 128-tiles)
SOFT_CAP = 50.0
MASK_BIAS = -1.0e30


def _band(qi, w_tiles):
    return list(range(max(0, qi - w_tiles), qi + 1))


def build_attention_nc(s_tiles=S_TILES, w_tiles=W_TILES, g=G, d=D, group=3):
    """Build the single-core Bass program (SPMD across 8 cores)."""
    s = s_tiles * 128
    qw = g * 128  # query columns per q-tile (all heads side by side)

    nc = bacc.Bacc("TRN2", target_bir_lowering=False, debug=False)

    q_dram = nc.dram_tensor("q", [s, g, d], F32R, kind="ExternalInput")
    k_dram = nc.dram_tensor("k", [s, d], F32R, kind="ExternalInput")
    v_dram = nc.dram_tensor("v", [s, d], F32R, kind="ExternalInput")
    ident_dram = nc.dram_tensor("ident", [128, 128], F32R, kind="ExternalInput")
    onesc_dram = nc.dram_tensor("onesc", [128, 1], F32R, kind="ExternalInput")
    onesr_dram = nc.dram_tensor("onesr", [1, 128], F32R, kind="ExternalInput")
    u1_dram = nc.dram_tensor("u1", [128, 128], F32R, kind="ExternalInput")
    u2_dram = nc.dram_tensor("u2", [128, 128], F32R, kind="ExternalInput")
    w1_dram = nc.dram_tensor("w1", [128, qw], F32R, kind="ExternalInput")
    w2_dram = nc.dram_tensor("w2", [128, qw], F32R, kind="ExternalInput")
    sel_dram = nc.dram_tensor(
        "sel", [s_tiles, s_tiles * 128], F32R, kind="ExternalInput"
    )
    out_dram = nc.dram_tensor("out", [s_tiles, d, qw], F32, kind="ExternalOutput")

    tanh_scale = 1.0 / (SOFT_CAP * math.sqrt(d))

    with tile.TileContext(nc) as tc:
        with ExitStack() as ctx:
            consts = ctx.enter_context(tc.tile_pool(name="consts", bufs=1))
            idt = consts.tile([128, 128], F32R, tag="idt")
            nc.sync.dma_start(idt[:], ident_dram.ap()[:])
            onesc = consts.tile([128, 1], F32R, tag="onesc")
            nc.sync.dma_start(onesc[:], onesc_dram.ap()[:])
            onesr = consts.tile([1, 128], F32R, tag="onesr")
            nc.sync.dma_start(onesr[:], onesr_dram.ap()[:])
            u1t = consts.tile([128, 128], F32R, tag="u1")
            nc.sync.dma_start(u1t[:], u1_dram.ap()[:])
            u2t = consts.tile([128, 128], F32R, tag="u2")
            nc.sync.dma_start(u2t[:], u2_dram.ap()[:])
            w1t = consts.tile([128, qw], F32R, tag="w1")
            nc.sync.dma_start(w1t[:], w1_dram.ap()[:])
            w2t = consts.tile([128, qw], F32R, tag="w2")
            nc.sync.dma_start(w2t[:], w2_dram.ap()[:])
            selt = consts.tile([s_tiles, s_tiles * 128], F32R, tag="sel")
            nc.sync.dma_start(selt[:], sel_dram.ap()[:])

            kt_pool = ctx.enter_context(tc.tile_pool(name="ktp", bufs=1))
            qt_pool = ctx.enter_context(tc.tile_pool(name="qtp", bufs=1))
            vv_pool = ctx.enter_context(tc.tile_pool(name="vvp", bufs=1))
            park_pool = ctx.enter_context(tc.tile_pool(name="parkp", bufs=1))
            dn_pool = ctx.enter_context(tc.tile_pool(name="dnp", bufs=1))

            vts = []
            for kj in range(s_tiles):
                vt = vv_pool.tile([128, d], F32R, tag=f"v{kj}", name=f"v{kj}")
                nc.sync.dma_start(vt[:], v_dram.ap()[kj * 128 : (kj + 1) * 128, :])
                vts.append(vt)

            # Prep: K^T and Q^T via PE transpose (PSUM) + copy to SBUF
            kts = []
            qts = []
            with tc.tile_pool(name="prepps", bufs=4, space="PSUM") as pp_pool, \
                 tc.tile_pool(name="stage", bufs=4) as stage_pool:
                for kj in range(s_tiles):
                    stk = stage_pool.tile([128, d], F32R, tag="stage", name=f"stk{kj}")
                    nc.sync.dma_start(stk[:], k_dram.ap()[kj * 128 : (kj + 1) * 128, :])
                    psk = pp_pool.tile([128, 128], F32R, tag="pp", name=f"psk{kj}")
                    nc.tensor.transpose(psk[:], stk[:], idt[:])
                    ktile = kt_pool.tile([128, 128], F32R, tag=f"kt{kj}", name=f"kt{kj}")
                    nc.vector.tensor_copy(ktile[:], psk[:])
                    kts.append(ktile)
                for qi in range(s_tiles):
                    qt = qt_pool.tile([128, qw], F32R, tag=f"qt{qi}", name=f"qt{qi}")
                    for gg in range(g):
                        stq = stage_pool.tile(
                            [128, d], F32R, tag="stage", name=f"stq{qi}_{gg}"
                        )
                        nc.sync.dma_start(
                            stq[:], q_dram.ap()[qi * 128 : (qi + 1) * 128, gg, :]
                        )
                        psq = pp_pool.tile(
                            [128, 128], F32R, tag="pp", name=f"psq{qi}_{gg}"
                        )
                        nc.tensor.transpose(psq[:], stq[:], idt[:])
                        nc.vector.tensor_copy(qt[:, gg * 128 : (gg + 1) * 128], psq[:])
                    qts.append(qt)

            park = park_pool.tile([128, s_tiles * qw], F32, tag="park")
            denom_sb = dn_pool.tile([s_tiles, qw], F32, tag="dsb")

            # Main banded attention loop
            with tc.tile_pool(name="lgp", bufs=2, space="PSUM") as lg_pool, \
                 tc.tile_pool(name="otp", bufs=1, space="PSUM") as ot_pool, \
                 tc.tile_pool(name="dnpp", bufs=1, space="PSUM") as dnp_pool, \
                 tc.tile_pool(name="pexp", bufs=2) as p_pool:
                for qi in range(s_tiles):
                    band = _band(qi, w_tiles)
                    first, last = band[0], band[-1]
                    ot = ot_pool.tile([128, qw], F32, tag="ot", name=f"ot{qi}")
                    dnt = dnp_pool.tile([1, qw], F32, tag="dn", name=f"dn{qi}")
                    for c0 in range(0, len(band), group):
                        chunk = band[c0 : c0 + group]
                        w = len(chunk) * qw
                        lg = lg_pool.tile(
                            [128, group * qw], F32, tag="lg", name=f"lg{qi}_{c0}"
                        )
                        for t, kj in enumerate(chunk):
                            sl = lg[:, t * qw : (t + 1) * qw]
                            is_diag = kj == qi
                            is_far = kj == qi - w_tiles
                            nc.tensor.matmul(
                                sl,
                                kts[kj][:],
                                qts[qi][:],
                                start=True,
                                stop=not (is_diag or is_far),
                            )
                            if is_diag:
                                nc.tensor.matmul(
                                    sl, u1t[:], w1t[:], start=False, stop=True
                                )
                            elif is_far:
                                nc.tensor.matmul(
                                    sl, u2t[:], w2t[:], start=False, stop=True
                                )
                        nc.scalar.activation(
                            lg[:, :w], lg[:, :w], AFT.Tanh, scale=tanh_scale
                        )
                        pt = p_pool.tile(
                            [128, group * qw], F32R, tag="p", name=f"p{qi}_{c0}"
                        )
                        nc.scalar.activation(
                            pt[:, :w], lg[:, :w], AFT.Exp, scale=SOFT_CAP
                        )
                        for t, kj in enumerate(chunk):
                            psl = pt[:, t * qw : (t + 1) * qw]
                            nc.tensor.matmul(
                                ot[:],
                                vts[kj][:],
                                psl,
                                start=(kj == first),
                                stop=(kj == last),
                            )
                            nc.tensor.matmul(
                                dnt[:],
                                onesc[:],
                                psl,
                                start=(kj == first),
                                stop=(kj == last),
                            )
                    nc.vector.tensor_copy(park[:, qi * qw : (qi + 1) * qw], ot[:])
                    dstage = p_pool.tile([1, qw], F32, tag="dst", name=f"dst{qi}")
                    nc.vector.tensor_copy(dstage[:], dnt[:])
                    nc.sync.dma_start(denom_sb[qi : qi + 1, :], dstage[:])

            # Finale: normalize all q-tiles
            recip = dn_pool.tile([s_tiles, qw], F32R, tag="recip")
            with nc.allow_low_precision(reason="f32r is f32-backed"):
                nc.vector.reciprocal(recip[:], denom_sb[:])
            with tc.tile_pool(name="rbp", bufs=2, space="PSUM") as rb_pool, \
                 tc.tile_pool(name="outp", bufs=2) as out_pool:
                for qi in range(s_tiles):
                    rb = rb_pool.tile([128, qw], F32, tag="rb", name=f"rb{qi}")
                    nc.tensor.matmul(
                        rb[:],
                        selt[:, qi * 128 : (qi + 1) * 128],
                        recip[:],
                        start=True,
                        stop=True,
                    )
                    ob = out_pool.tile([128, qw], F32, tag="ob", name=f"ob{qi}")
                    nc.vector.tensor_mul(
                        ob[:], park[:, qi * qw : (qi + 1) * qw], rb[:]
                    )
                    nc.sync.dma_start(out_dram.ap()[qi], ob[:])

    nc.compile()
    return nc


def make_const_inputs(g=G, qw=None, s_tiles=S_TILES):
    if qw is None:
        qw = g * 128
    r = np.arange(128)
    ident = np.eye(128, dtype=np.float32)
    onesc = np.ones((128, 1), dtype=np.float32)
    onesr = np.ones((1, 128), dtype=np.float32)
    # u1[k, r] = 1 if k <= r ; w1[k, col] = MASK_BIAS if k > (col % 128)
    u1 = (r[:, None] <= r[None, :]).astype(np.float32)
    u2 = (r[:, None] >= r[None, :]).astype(np.float32)
    c = np.tile(r, qw // 128)
    w1 = np.where(r[:, None] > c[None, :], np.float32(MASK_BIAS), np.float32(0.0))
    w2 = np.where(r[:, None] <= c[None, :], np.float32(MASK_BIAS), np.float32(0.0))
    sel = np.zeros((s_tiles, s_tiles * 128), dtype=np.float32)
    for qi in range(s_tiles):
        sel[qi, qi * 128 : (qi + 1) * 128] = 1.0
    return {
        "sel": sel,
        "ident": ident,
        "onesc": onesc,
        "onesr": onesr,
        "u1": u1,
        "u2": u2,
        "w1": np.ascontiguousarray(w1.astype(np.float32)),
        "w2": np.ascontiguousarray(w2.astype(np.float32)),
    }


def shard_inputs(query, key, value):
    """Split full [B,S,NQ,D]/[B,S,NKV,D] inputs into 8 per-core maps."""
    consts = make_const_inputs()
    in_maps = []
    for b in range(B):
        for h in range(NKV):
            m = dict(consts)
            m["q"] = np.ascontiguousarray(
                query[b, :, h * G : (h + 1) * G, :], dtype=np.float32
            )
            m["k"] = np.ascontiguousarray(key[b, :, h, :], dtype=np.float32)
            m["v"] = np.ascontiguousarray(value[b, :, h, :], dtype=np.float32)
            in_maps.append(m)
    return in_maps


def gather_output(results):
    """Per-core "out" [S_TILES, D, G*128] -> full [B, S, NQ, D]."""
    full = np.empty((B, S, NQ, D), dtype=np.float32)
    for b in range(B):
        for h in range(NKV):
            o = results[b * NKV + h]["out"]
            # [qi, d, g*128+c] -> [qi, c, g, d] -> [S, G, D]
            o = o.reshape(S_TILES, D, G, 128).transpose(0, 3, 2, 1)
            full[b, :, h * G : (h + 1) * G, :] = o.reshape(S, G, D)
    return full


_NC_CACHE = {}


def _get_nc():
    if "nc" not in _NC_CACHE:
        _NC_CACHE["nc"] = build_attention_nc()
    return _NC_CACHE["nc"]


def kernel(query, key, value, decoder_segment_ids=None, **_unused):
    query = np.asarray(query, dtype=np.float32)
    key = np.asarray(key, dtype=np.float32)
    value = np.asarray(value, dtype=np.float32)
    nc = _get_nc()
    in_maps = shard_inputs(query, key, value)
    res = run_bass_kernel_spmd(nc, in_maps, core_ids=list(range(8)))
    return gather_output(res.results)


if __name__ == "__main__":
    rng = np.random.default_rng(0)
    q = rng.standard_normal((B, S, NQ, D), dtype=np.float32)
    k = rng.standard_normal((B, S, NKV, D), dtype=np.float32)
    v = rng.standard_normal((B, S, NKV, D), dtype=np.float32)
    seg = np.ones((B, S), dtype=np.int32)
    out = kernel(query=q, key=k, value=v, decoder_segment_ids=seg)
    print(out.shape, out.dtype, float(np.abs(out).max()))


# revision 22
# speedup vs baseline: 467.2450x; 467.2450x over previous
"""Sliding-window GQA attention (maxtext-style) on 8 Trainium2 NeuronCores.

Problem (hardcoded): B=4, S=2048, NQ=8, NKV=2, D=128, window=1024,
logit soft-cap 50, causal. decoder_segment_ids is all-ones per the input
spec, so the segment mask reduces to causal+window and is not computed on
device.

Sharding: one core per (batch b, kv-head h) pair -> 8 cores, no
collectives. Each core runs sliding-window flash attention for its 4
query heads against its single shared K/V head.

Per-core layout ("layout B"): logits are computed transposed,
L[s, q] = (K Q^T)^T tiles, so the exp'd probabilities P[s, q] feed the
P->V matmul directly as the moving operand (lhsT = V[s, d] natural,
out = O^T[d, q]) with no per-tile P transposes. Softmax needs no
max-subtraction because the tanh soft-cap bounds logits to +-50.
Band masking (causal diagonal + far window edge) is applied by
accumulating a -1e30 rank-128 bias product into the logits PSUM, which
the tanh saturates to -1 -> exp gives e^-50 ~ 2e-22 (negligible).
Row sums ride on a [1, q] ones-matmul accumulated alongside O^T; the
final normalize is a reciprocal + broadcast-matmul + vector multiply.
"""

import math
from contextlib import ExitStack

import numpy as np

import concourse.bass as bass
import concourse.tile as tile
from concourse import bacc, mybir
from concourse.bass_utils import run_bass_kernel_spmd

F32 = mybir.dt.float32
F32R = mybir.dt.float32r
AFT = mybir.ActivationFunctionType

# Full-size problem constants
B, S, NQ, NKV, D = 4, 2048, 8, 2, 128
G = NQ // NKV  # 4 query heads per kv head
S_TILES = S // 128  # 16
W_TILES = 1024 // 128  # 8 (sliding window in 128-tiles)
SOFT_CAP = 50.0
MASK_BIAS = -1.0e30


def _band(qi, w_tiles):
    return list(range(max(0, qi - w_tiles), qi + 1))


def build_attention_nc(s_tiles=S_TILES, w_tiles=W_TILES, g=G, d=D, group=2):
    """Build the single-core Bass program (SPMD across 8 cores)."""
    s = s_tiles * 128
    qw = g * 128  # query columns per q-tile (all heads side by side)

    nc = bacc.Bacc("TRN2", target_bir_lowering=False, debug=False)

    q_dram = nc.dram_tensor("q", [s, g, d], F32R, kind="ExternalInput")
    k_dram = nc.dram_tensor("k", [s, d], F32R, kind="ExternalInput")
    v_dram = nc.dram_tensor("v", [s, d], F32R, kind="ExternalInput")
    ident_dram = nc.dram_tensor("ident", [128, 128], F32R, kind="ExternalInput")
    onesc_dram = nc.dram_tensor("onesc", [128, 1], F32R, kind="ExternalInput")
    onesr_dram = nc.dram_tensor("onesr", [1, 128], F32R, kind="ExternalInput")
    u1_dram = nc.dram_tensor("u1", [128, 128], F32R, kind="ExternalInput")
    u2_dram = nc.dram_tensor("u2", [128, 128], F32R, kind="ExternalInput")
    w1_dram = nc.dram_tensor("w1", [128, qw], F32R, kind="ExternalInput")
    w2_dram = nc.dram_tensor("w2", [128, qw], F32R, kind="ExternalInput")
    sel_dram = nc.dram_tensor(
        "sel", [s_tiles, s_tiles * 128], F32R, kind="ExternalInput"
    )
    out_dram = nc.dram_tensor("out", [s_tiles, d, qw], F32, kind="ExternalOutput")

    tanh_scale = 1.0 / (SOFT_CAP * math.sqrt(d))

    # Normalize batches: (q-tiles, trigger after emit_main_qi(trigger_qi));
    # trigger None = tail. A batch's denominators are all staged once
    # main(last_qi_of_batch + 2) has been emitted.
    if s_tiles >= 8:
        batches = [
            (list(range(0, s_tiles // 2)), s_tiles // 2 + 1),
            (list(range(s_tiles // 2, s_tiles - 2)), s_tiles - 1),
            ([s_tiles - 2, s_tiles - 1], None),
        ]
    else:
        batches = [(list(range(s_tiles)), None)]

    with tile.TileContext(nc) as tc:
        with ExitStack() as ctx:
            consts = ctx.enter_context(tc.tile_pool(name="consts", bufs=1))
            idt = consts.tile([128, 128], F32R, tag="idt")
            nc.sync.dma_start(idt[:], ident_dram.ap()[:])
            onesc = consts.tile([128, 1], F32R, tag="onesc")
            nc.sync.dma_start(onesc[:], onesc_dram.ap()[:])
            u1t = consts.tile([128, 128], F32R, tag="u1")
            nc.sync.dma_start(u1t[:], u1_dram.ap()[:])
            u2t = consts.tile([128, 128], F32R, tag="u2")
            nc.sync.dma_start(u2t[:], u2_dram.ap()[:])
            w1t = consts.tile([128, qw], F32R, tag="w1")
            nc.sync.dma_start(w1t[:], w1_dram.ap()[:])
            w2t = consts.tile([128, qw], F32R, tag="w2")
            nc.sync.dma_start(w2t[:], w2_dram.ap()[:])
            selt = consts.tile([s_tiles, s_tiles * 128], F32R, tag="sel")
            nc.sync.dma_start(selt[:], sel_dram.ap()[:])

            kt_pool = ctx.enter_context(tc.tile_pool(name="ktp", bufs=1))
            qt_pool = ctx.enter_context(tc.tile_pool(name="qtp", bufs=1))
            vv_pool = ctx.enter_context(tc.tile_pool(name="vvp", bufs=1))
            park_pool = ctx.enter_context(tc.tile_pool(name="parkp", bufs=1))
            dn_pool = ctx.enter_context(tc.tile_pool(name="dnp", bufs=1))
            stage_pool = ctx.enter_context(tc.tile_pool(name="stagep", bufs=1))
            p_pool = ctx.enter_context(tc.tile_pool(name="pexp", bufs=2))
            out_pool = ctx.enter_context(tc.tile_pool(name="outp", bufs=2))

            # Bulk loads on gpsimd (SWDGE) so the SP queue stays free;
            # chunked + interleaved in need-order so early tiles unblock fast
            vv = vv_pool.tile([128, s_tiles * d], F32R, tag="vv")
            stage_k = stage_pool.tile([128, s_tiles * d], F32R, tag="stk")
            stage_q = stage_pool.tile([128, s_tiles * g * d], F32R, tag="stq")

            def dma_k_chunk(t0, t1):
                nc.gpsimd.dma_start(
                    stage_k[:, t0 * d : t1 * d].rearrange("p (t d) -> p t d", d=d),
                    k_dram.ap()[t0 * 128 : t1 * 128, :].rearrange(
                        "(t p) d -> p t d", p=128
                    ),
                )

            def dma_v_chunk(t0, t1):
                nc.gpsimd.dma_start(
                    vv[:, t0 * d : t1 * d].rearrange("p (t d) -> p t d", d=d),
                    v_dram.ap()[t0 * 128 : t1 * 128, :].rearrange(
                        "(t p) d -> p t d", p=128
                    ),
                )

            def dma_q_chunk(t0, t1):
                nc.gpsimd.dma_start(
                    stage_q[:, t0 * g * d : t1 * g * d].rearrange(
                        "p (t g d) -> p t g d", g=g, d=d
                    ),
                    q_dram.ap()[t0 * 128 : t1 * 128, :, :].rearrange(
                        "(t p) g d -> p t g d", p=128
                    ),
                )

            kc = max(1, s_tiles // 4)
            qc = max(1, s_tiles // 8)
            ev = []
            for i in range(s_tiles // kc):
                ev.append((dma_k_chunk, i * kc, (i + 1) * kc))
                ev.append((dma_v_chunk, i * kc, (i + 1) * kc))
            evq = [
                (dma_q_chunk, i * qc, (i + 1) * qc) for i in range(s_tiles // qc)
            ]
            order = []
            qi_ = 0
            for i, e in enumerate(ev):
                order.append(e)
                while qi_ < len(evq) and len(order) % 2 == 1:
                    order.append(evq[qi_])
                    qi_ += 1
            order.extend(evq[qi_:])
            for fn, a, b in order:
                fn(a, b)

            park = park_pool.tile([128, s_tiles * qw], F32, tag="park")
            # per-batch denominator staging + reciprocal tiles (all base-0)
            dsbs = {}
            recips = {}
            qi2batch = {}
            for bi, (qis, _trig) in enumerate(batches):
                dsbs[bi] = dn_pool.tile(
                    [len(qis), qw], F32, tag=f"dsb{bi}", name=f"dsb{bi}"
                )
                recips[bi] = dn_pool.tile(
                    [len(qis), qw], F32R, tag=f"recip{bi}", name=f"recip{bi}"
                )
                for r, qi in enumerate(qis):
                    qi2batch[qi] = (bi, r)

            # PSUM banks (8): prep 2 + lg 2x2 + ot 1 + dn 1
            with tc.tile_pool(name="prepps", bufs=2, space="PSUM") as pp_pool, \
                 tc.tile_pool(name="lgp", bufs=2, space="PSUM") as lg_pool, \
                 tc.tile_pool(name="otp", bufs=1, space="PSUM") as ot_pool, \
                 tc.tile_pool(name="dnpp", bufs=1, space="PSUM") as dnp_pool:
                kts = [None] * s_tiles
                qts = [None] * s_tiles
                ots = {}
                dnts = {}
                state = {"pending": None}

                def emit_prep(i):
                    psk = pp_pool.tile([128, 128], F32R, tag="pp", name=f"psk{i}")
                    nc.tensor.transpose(
                        psk[:], stage_k[:, i * d : (i + 1) * d], idt[:]
                    )
                    ktile = kt_pool.tile(
                        [128, 128], F32R, tag=f"kt{i}", name=f"kt{i}"
                    )
                    nc.vector.tensor_copy(ktile[:], psk[:])
                    kts[i] = ktile
                    qt = qt_pool.tile([128, qw], F32R, tag=f"qt{i}", name=f"qt{i}")
                    for gg in range(g):
                        psq = pp_pool.tile(
                            [128, 128], F32R, tag="pp", name=f"psq{i}_{gg}"
                        )
                        nc.tensor.transpose(
                            psq[:],
                            stage_q[:, (i * g + gg) * d : (i * g + gg + 1) * d],
                            idt[:],
                        )
                        nc.vector.tensor_copy(qt[:, gg * 128 : (gg + 1) * 128], psq[:])
                    qts[i] = qt

                def emit_pv(qi, band, chunk, pt, last_chunk):
                    first, last = band[0], band[-1]
                    for t, kj in enumerate(chunk):
                        psl = pt[:, t * qw : (t + 1) * qw]
                        nc.tensor.matmul(
                            ots[qi][:],
                            vv[:, kj * d : (kj + 1) * d],
                            psl,
                            start=(kj == first),
                            stop=(kj == last),
                        )
                        nc.tensor.matmul(
                            dnts[qi][:],
                            onesc[:],
                            psl,
                            start=(kj == first),
                            stop=(kj == last),
                        )
                    if last_chunk:
                        nc.vector.tensor_copy(
                            park[:, qi * qw : (qi + 1) * qw], ots[qi][:]
                        )
                        dstage = p_pool.tile([1, qw], F32, tag="dst", name=f"dst{qi}")
                        nc.vector.tensor_copy(dstage[:], dnts[qi][:])
                        bi, r = qi2batch[qi]
                        nc.sync.dma_start(dsbs[bi][r : r + 1, :], dstage[:])

                def emit_main_qi(qi):
                    band = _band(qi, w_tiles)
                    ots[qi] = ot_pool.tile([128, qw], F32, tag="ot", name=f"ot{qi}")
                    dnts[qi] = dnp_pool.tile([1, qw], F32, tag="dn", name=f"dn{qi}")
                    for c0 in range(0, len(band), group):
                        chunk = band[c0 : c0 + group]
                        w = len(chunk) * qw
                        lg = lg_pool.tile(
                            [128, group * qw], F32, tag="lg", name=f"lg{qi}_{c0}"
                        )
                        for t, kj in enumerate(chunk):
                            sl = lg[:, t * qw : (t + 1) * qw]
                            is_diag = kj == qi
                            is_far = kj == qi - w_tiles
                            nc.tensor.matmul(
                                sl,
                                kts[kj][:],
                                qts[qi][:],
                                start=True,
                                stop=not (is_diag or is_far),
                            )
                            if is_diag:
                                nc.tensor.matmul(
                                    sl, u1t[:], w1t[:], start=False, stop=True
                                )
                            elif is_far:
                                nc.tensor.matmul(
                                    sl, u2t[:], w2t[:], start=False, stop=True
                                )
                        nc.scalar.activation(
                            lg[:, :w], lg[:, :w], AFT.Tanh, scale=tanh_scale
                        )
                        pt = p_pool.tile(
                            [128, group * qw], F32R, tag="p", name=f"p{qi}_{c0}"
                        )
                        nc.scalar.activation(
                            pt[:, :w], lg[:, :w], AFT.Exp, scale=SOFT_CAP
                        )
                        if state["pending"] is not None:
                            emit_pv(*state["pending"])
                        state["pending"] = (
                            qi,
                            band,
                            chunk,
                            pt,
                            c0 + group >= len(band),
                        )

                def emit_recip(bi):
                    with nc.allow_low_precision(reason="f32r is f32-backed"):
                        nc.vector.reciprocal(recips[bi][:], dsbs[bi][:])

                def emit_norm_pair(bi, pair, psum_pool, ptag):
                    qis, _trig = batches[bi]
                    rows = len(qis)
                    rbm = psum_pool.tile(
                        [128, group * qw], F32, tag=ptag, name=f"rbm{pair[0]}"
                    )
                    for j, qi in enumerate(pair):
                        r = qi - qis[0]
                        nc.tensor.matmul(
                            rbm[:, j * qw : (j + 1) * qw],
                            selt[0:rows, r * 128 : (r + 1) * 128],
                            recips[bi][:],
                            start=True,
                            stop=True,
                        )
                    nb = len(pair)
                    ob = out_pool.tile(
                        [128, 2 * qw], F32, tag="ob", name=f"ob{pair[0]}"
                    )
                    nc.vector.tensor_mul(
                        ob[:, : nb * qw],
                        park[:, pair[0] * qw : (pair[0] + nb) * qw],
                        rbm[:, : nb * qw],
                    )
                    nc.sync.dma_start(
                        out_dram.ap()[pair[0] : pair[0] + nb].rearrange(
                            "t p c -> p t c"
                        ),
                        ob[:, : nb * qw].rearrange("p (t c) -> p t c", t=nb),
                    )

                def emit_norm_batch(bi, psum_pool, ptag, with_recip=True):
                    if with_recip:
                        emit_recip(bi)
                    qis, _trig = batches[bi]
                    for b0 in range(0, len(qis), 2):
                        emit_norm_pair(bi, qis[b0 : b0 + 2], psum_pool, ptag)

                # Interleaved emission: prep(i) one q-tile ahead of main(i-1);
                # normalize work spread across hook points to avoid bursts
                hooks = {}
                if s_tiles >= 8:
                    b0_qis = batches[0][0]
                    hooks.setdefault(batches[0][1] - 1, []).append(
                        lambda: emit_recip(0)
                    )
                    for j in range(0, len(b0_qis), 2):
                        m = batches[0][1] + j // 2
                        pr = b0_qis[j : j + 2]
                        hooks.setdefault(m, []).append(
                            lambda pr=pr: emit_norm_pair(0, pr, lg_pool, "lg")
                        )
                    b1_qis = batches[1][0]
                    hooks.setdefault(batches[1][1] - 1, []).append(
                        lambda: emit_recip(1)
                    )
                    for j in range(0, len(b1_qis), 2):
                        pr = b1_qis[j : j + 2]
                        hooks.setdefault(batches[1][1], []).append(
                            lambda pr=pr: emit_norm_pair(1, pr, lg_pool, "lg")
                        )

                def run_hooks(m):
                    for fn in hooks.get(m, []):
                        fn()

                for i in range(s_tiles):
                    emit_prep(i)
                    if i >= 1:
                        emit_main_qi(i - 1)
                        run_hooks(i - 1)
                emit_main_qi(s_tiles - 1)
                run_hooks(s_tiles - 1)
                emit_pv(*state["pending"])
                state["pending"] = None

            # Tail: remaining batches
            with tc.tile_pool(name="rbp", bufs=2, space="PSUM") as rb_pool:
                for bi, (qis, trig) in enumerate(batches):
                    if trig is None:
                        emit_norm_batch(bi, rb_pool, "rb", with_recip=True)

    nc.compile()
    return nc


def make_const_inputs(g=G, qw=None, s_tiles=S_TILES):
    if qw is None:
        qw = g * 128
    r = np.arange(128)
    ident = np.eye(128, dtype=np.float32)
    onesc = np.ones((128, 1), dtype=np.float32)
    onesr = np.ones((1, 128), dtype=np.float32)
    # u1[k, r] = 1 if k <= r ; w1[k, col] = MASK_BIAS if k > (col % 128)
    u1 = (r[:, None] <= r[None, :]).astype(np.float32)
    u2 = (r[:, None] >= r[None, :]).astype(np.float32)
    c = np.tile(r, qw // 128)
    w1 = np.where(r[:, None] > c[None, :], np.float32(MASK_BIAS), np.float32(0.0))
    w2 = np.where(r[:, None] <= c[None, :], np.float32(MASK_BIAS), np.float32(0.0))
    sel = np.zeros((s_tiles, s_tiles * 128), dtype=np.float32)
    for qi in range(s_tiles):
        sel[qi, qi * 128 : (qi + 1) * 128] = 1.0
    return {
        "sel": sel,
        "ident": ident,
        "onesc": onesc,
        "onesr": onesr,
        "u1": u1,
        "u2": u2,
        "w1": np.ascontiguousarray(w1.astype(np.float32)),
        "w2": np.ascontiguousarray(w2.astype(np.float32)),
    }


def shard_inputs(query, key, value):
    """Split full [B,S,NQ,D]/[B,S,NKV,D] inputs into 8 per-core maps."""
    consts = make_const_inputs()
    in_maps = []
    for b in range(B):
        for h in range(NKV):
            m = dict(consts)
            m["q"] = np.ascontiguousarray(
                query[b, :, h * G : (h + 1) * G, :], dtype=np.float32
            )
            m["k"] = np.ascontiguousarray(key[b, :, h, :], dtype=np.float32)
            m["v"] = np.ascontiguousarray(value[b, :, h, :], dtype=np.float32)
            in_maps.append(m)
    return in_maps


def gather_output(results):
    """Per-core "out" [S_TILES, D, G*128] -> full [B, S, NQ, D]."""
    full = np.empty((B, S, NQ, D), dtype=np.float32)
    for b in range(B):
        for h in range(NKV):
            o = results[b * NKV + h]["out"]
            # [qi, d, g*128+c] -> [qi, c, g, d] -> [S, G, D]
            o = o.reshape(S_TILES, D, G, 128).transpose(0, 3, 2, 1)
            full[b, :, h * G : (h + 1) * G, :] = o.reshape(S, G, D)
    return full


_NC_CACHE = {}


def _get_nc():
    if "nc" not in _NC_CACHE:
        _NC_CACHE["nc"] = build_attention_nc()
    return _NC_CACHE["nc"]


def kernel(query, key, value, decoder_segment_ids=None, **_unused):
    query = np.asarray(query, dtype=np.float32)
    key = np.asarray(key, dtype=np.float32)
    value = np.asarray(value, dtype=np.float32)
    nc = _get_nc()
    in_maps = shard_inputs(query, key, value)
    res = run_bass_kernel_spmd(nc, in_maps, core_ids=list(range(8)))
    return gather_output(res.results)


if __name__ == "__main__":
    rng = np.random.default_rng(0)
    q = rng.standard_normal((B, S, NQ, D), dtype=np.float32)
    k = rng.standard_normal((B, S, NKV, D), dtype=np.float32)
    v = rng.standard_normal((B, S, NKV, D), dtype=np.float32)
    seg = np.ones((B, S), dtype=np.int32)
    out = kernel(query=q, key=k, value=v, decoder_segment_ids=seg)
    print(out.shape, out.dtype, float(np.abs(out).max()))
